# revision 10
# baseline (speedup 1.0000x reference)
"""FourierCrossAttention Trainium2 kernel, v5.

Sharding: one head per NeuronCore (H=8, n_cores=8).

Same numerics as the proven baseline (hi/lo fp16 DFT + e1, fp32 tanh
chain), restructured for pipelining: per-quad (4-batch) tanh/e2/wmix and
per-pair (2-batch) inverse-DFT/output so the tail after the last input
DMA is short. Inputs on sync/HWDGE DMA, outputs on scalar DMA, PE warmed
up with a dummy matmul chain during the first input loads.
"""
import sys
sys.path.insert(0, '/opt/trn_rl_repo')
import numpy as np
from contextlib import ExitStack

import concourse.bacc as bacc
import concourse.mybir as mybir
import concourse.tile as tile
from concourse.bass_utils import run_bass_kernel_spmd

F32 = mybir.dt.float32
F16 = mybir.dt.float16
AF = mybir.ActivationFunctionType
ALU = mybir.AluOpType

B, E, H, L = 16, 64, 8, 2048
M = 64
NCH = 16
S_W = 4096.0
S_X = 2.0 ** -18
S_OUT = 2.0 ** 14
S_C = 2.0 ** -11
S_HOST = 2.0 ** -20
# S_W * S_X * S_C * S_OUT = 2^-23 = 1/(L*E*E)

INV_PI = float(1.0 / np.pi)
MAGIC = 12582912.0
C1H = 3.140625              # Cody-Waite pi hi (exact in 9 bits)
C2WH = float(np.pi - 3.140625)
HALF_PI = float(np.pi / 2)

N_WARM = 10

_CACHE = {}


def _dft_consts():
    l = np.arange(L)[:, None]
    m = np.arange(M)[None, :]
    ang = 2.0 * np.pi * l * m / L
    c2 = np.concatenate([np.cos(ang), -np.sin(ang)], axis=1)   # [L, 128]
    c2hi = c2.astype(np.float16)
    c2lo = (c2 - c2hi.astype(np.float64)).astype(np.float16)
    c2hi = c2hi.reshape(128, NCH, 128)
    c2lo = c2lo.reshape(128, NCH, 128)

    x = np.arange(M)[:, None]
    lr = np.arange(L)[None, :]
    ang2 = 2.0 * np.pi * x * lr / L
    dup = np.where(np.arange(M) == 0, 1.0, 2.0)[:, None]
    cr = dup * np.cos(ang2) * (S_C * S_OUT)
    ci = -dup * np.sin(ang2) * (S_C * S_OUT)
    ci[0, :] = 0.0
    cinv = np.concatenate([cr, ci], axis=0).astype(np.float16)  # [128, L]

    idd = np.zeros((128, 64), np.float16)
    idd[64:128, :] = np.eye(64, dtype=np.float16)
    i128 = np.eye(128, dtype=np.float16)
    return c2hi, c2lo, cinv, idd, i128


def _patch_act_tables():
    """Steer the act-table placement pass to a single table that contains
    every function we use (Copy/Tanh/Sin/Square), keeping dict order/size
    intact so act_func_set_id indices still match act_info.json."""
    if getattr(bacc, "_act_tables_patched", False):
        return
    orig = bacc.get_activation_tables
    need = {AF.Copy, AF.Tanh, AF.Sin, AF.Square}

    def steered(arch):
        tabs = orig(arch)
        if any(need <= v for v in tabs.values()):
            return {k: (v if need <= v else set()) for k, v in tabs.items()}
        return tabs

    bacc.get_activation_tables = steered
    bacc._act_tables_patched = True


def _build():
    _patch_act_tables()
    nc = bacc.Bacc("TRN2", target_bir_lowering=False, debug=False)

    def reg_const(value, dtype=F32):
        t = nc.alloc_sbuf_tensor(f"const-{dtype.name}-{value}", [128, 1], dtype)
        nc.gpsimd.memset(t.ap(), value)
        nc.const_aps.aps[(dtype, value)] = t.ap()

    reg_const(HALF_PI)
    nc.all_engine_barrier()

    c2hi_np, c2lo_np, cinv_np, idd_np, i128_np = _dft_consts()
    C2H = nc.inline_tensor(np.ascontiguousarray(c2hi_np), name="C2H")
    C2L = nc.inline_tensor(np.ascontiguousarray(c2lo_np), name="C2L")
    CINV = nc.inline_tensor(np.ascontiguousarray(cinv_np), name="CINV")
    IDD = nc.inline_tensor(np.ascontiguousarray(idd_np), name="IDD")
    I128 = nc.inline_tensor(np.ascontiguousarray(i128_np), name="I128")

    QK = nc.dram_tensor("qk", [B, 128, 2, NCH, 128], F16, kind="ExternalInput")
    WP = nc.dram_tensor("wp", [64, 3, M, 64], F16, kind="ExternalInput")
    OUT = nc.dram_tensor("out", [B, E, L], F16, kind="ExternalOutput")

    with tile.TileContext(nc) as tc, ExitStack() as ctx:
        cpool = ctx.enter_context(tc.tile_pool(name="consts", bufs=1))
        qk_pool = ctx.enter_context(tc.tile_pool(name="qk", bufs=6))
        ft_pool = ctx.enter_context(tc.tile_pool(name="ft", bufs=4))
        ktr_pool = ctx.enter_context(tc.tile_pool(name="ktr", bufs=4))
        th_pool = ctx.enter_context(tc.tile_pool(name="tanh", bufs=4))
        pk_pool = ctx.enter_context(tc.tile_pool(name="pack", bufs=4))
        st_pool = ctx.enter_context(tc.tile_pool(name="stage", bufs=5))
        ot_pool = ctx.enter_context(tc.tile_pool(name="ot", bufs=5))
        ps_f = ctx.enter_context(tc.tile_pool(name="ps_f", bufs=2,
                                              space="PSUM"))
        ps_e1 = ctx.enter_context(tc.tile_pool(name="ps_e1", bufs=2,
                                               space="PSUM"))
        ps_vx = ctx.enter_context(tc.tile_pool(name="ps_vx", bufs=2,
                                               space="PSUM"))
        ps_o = ctx.enter_context(tc.tile_pool(name="ps_o", bufs=2,
                                              space="PSUM"))

        # ---------------- constants + warm-up ----------------
        idd = cpool.tile([128, 64], F16)
        nc.sync.dma_start(idd[:], IDD[:])
        i128 = cpool.tile([128, 128], F16)
        nc.sync.dma_start(i128[:], I128[:])
        c2h = cpool.tile([128, NCH, 128], F16)
        nc.sync.dma_start(c2h[:], C2H[:])
        c2l = cpool.tile([128, NCH, 128], F16)
        nc.sync.dma_start(c2l[:], C2L[:])
        cinv = cpool.tile([128, L], F16)
        wp = cpool.tile([64, 3, M, 64], F16)

        wtile = cpool.tile([128, 512], F16)
        nc.gpsimd.memset(wtile[:], 0.25)
        ps_warm = ps_o.tile([128, 512], F32, tag="po", name="ps_warm")
        for i in range(N_WARM):
            nc.tensor.matmul(ps_warm[:], wtile[:, 0:128], wtile[:],
                             start=(i == 0), stop=(i == N_WARM - 1))

        ftHLs = [None] * 4
        ftq2s = [None] * 4
        ftqNs = [None] * 4
        ktr4s = [None] * 4
        e1_tiles = [None] * 2
        pk_tiles = [None] * 4
        xs_tiles = [None] * 4

        def load_qk(b):
            qkt = qk_pool.tile([128, 2, NCH, 128], F16, tag="qkt")
            nc.sync.dma_start(qkt[:], QK[b])
            return qkt

        qk_tiles = {}
        next_load = [0]

        def load_ahead(n):
            while next_load[0] < B and next_load[0] < n:
                qk_tiles[next_load[0]] = load_qk(next_load[0])
                next_load[0] += 1

        def emit_quad(quad):
            """DFT + ft staging + ktr + e1 for batches 4q .. 4q+3."""
            g, q = divmod(quad, 2)
            b0 = quad * 4
            pf = ps_f.tile([128, 4, 128], F32, tag="pf", name=f"pf{quad}")
            for i in range(4):
                qkt = qk_tiles.pop(b0 + i)
                qh = qkt[:, 0]
                ql = qkt[:, 1]
                for n in range(NCH):
                    nc.tensor.matmul(pf[:, i], qh[:, n], c2h[:, n, :],
                                     start=(n == 0), stop=False)
                    nc.tensor.matmul(pf[:, i], qh[:, n], c2l[:, n, :],
                                     start=False, stop=False)
                    nc.tensor.matmul(pf[:, i], ql[:, n], c2h[:, n, :],
                                     start=False, stop=(n == NCH - 1))

            ftHL = ft_pool.tile([128, 2, 4, 128], F16, tag="ftHL")
            nc.scalar.copy(ftHL[:, 0], pf[:])
            nc.vector.tensor_sub(ftHL[:, 1], pf[:], ftHL[:, 0])
            ftq2 = ft_pool.tile([128, 2, 4, 128], F16, tag="ftq2")
            nc.vector.tensor_copy(ftq2[64:128], ftHL[0:64])
            ftqN = ft_pool.tile([128, 2, 4, 64], F16, tag="ftqN")
            nc.vector.tensor_scalar_mul(ftqN[64:128],
                                        ftHL[0:64, :, :, 64:128], -1.0)
            ftHLs[quad] = ftHL
            ftq2s[quad] = ftq2
            ftqNs[quad] = ftqN

            pkt = ps_vx.tile([128, 4, 64], F16, tag="vx", name=f"pkt{quad}")
            for i in range(4):
                nc.tensor.transpose(pkt[:, i, :], ftHL[64:128, 0, i, :],
                                    idd[64:128, :])
            ktr4 = ktr_pool.tile([128, 4, 64], F16, tag="ktr4")
            nc.vector.tensor_copy(ktr4[:], pkt[:])
            ktr4s[quad] = ktr4

            ps_re, ps_im = e1_tiles[g]
            for i in range(4):
                col = (q * 4 + i) * 64
                KreH = ftHL[64:128, 0, i, 0:64]
                KimH = ftHL[64:128, 0, i, 64:128]
                KreL = ftHL[64:128, 1, i, 0:64]
                KimL = ftHL[64:128, 1, i, 64:128]
                QreH = ftq2[64:128, 0, i, 0:64]
                QimH = ftq2[64:128, 0, i, 64:128]
                QreL = ftq2[64:128, 1, i, 0:64]
                QimL = ftq2[64:128, 1, i, 64:128]
                nQimH = ftqN[64:128, 0, i, :]
                nQimL = ftqN[64:128, 1, i, :]
                o_re = ps_re[:, col:col + 64]
                o_im = ps_im[:, col:col + 64]
                nc.tensor.matmul(o_re, KreH, QreH, start=True, stop=False)
                nc.tensor.matmul(o_re, KreH, QreL, start=False, stop=False)
                nc.tensor.matmul(o_im, KreH, QimH, start=True, stop=False)
                nc.tensor.matmul(o_im, KreH, QimL, start=False, stop=False)
                nc.tensor.matmul(o_re, KimH, nQimH, start=False, stop=False)
                nc.tensor.matmul(o_re, KimH, nQimL, start=False, stop=False)
                nc.tensor.matmul(o_im, KimH, QreH, start=False, stop=False)
                nc.tensor.matmul(o_im, KimH, QreL, start=False, stop=False)
                nc.tensor.matmul(o_re, KreL, QreH, start=False, stop=False)
                nc.tensor.matmul(o_im, KreL, QimH, start=False, stop=False)
                nc.tensor.matmul(o_re, KimL, nQimH, start=False, stop=True)
                nc.tensor.matmul(o_im, KimL, QreH, start=False, stop=True)

        def emit_tanh(quad):
            """Complex tanh for one quad's [64, 256] slice of the e1 psum."""
            g, q = divmod(quad, 2)
            ps_re, ps_im = e1_tiles[g]
            c0, c1 = q * 256, (q + 1) * 256
            p_re = ps_re[:, c0:c1]
            p_im = ps_im[:, c0:c1]
            tb = lambda dt=F32, tag="tt": th_pool.tile(
                [64, 256], dt, tag=tag, name=f"th_{tag}_{quad}")
            t = tb(tag="t")
            nc.scalar.activation(t[:], p_re, AF.Tanh)
            u1 = tb(tag="u1")
            nc.vector.tensor_scalar(u1[:], p_im, INV_PI, MAGIC,
                                    ALU.mult, ALU.add)
            w1 = tb(tag="w1")
            nc.gpsimd.tensor_scalar(w1[:], u1[:], MAGIC, C1H,
                                    ALU.subtract, ALU.mult)
            w2 = tb(tag="w2")
            nc.gpsimd.tensor_scalar(w2[:], u1[:], MAGIC, C2WH,
                                    ALU.subtract, ALU.mult)
            zr0 = tb(tag="zr0")
            nc.vector.tensor_sub(zr0[:], p_im, w1[:])
            zrh = tb(tag="zrh")
            nc.vector.tensor_sub(zrh[:], zr0[:], w2[:])
            s2 = tb(F16, tag="s2")
            nc.scalar.activation(s2[:], zrh[:], AF.Sin, scale=2.0)
            sh = tb(tag="sh")
            nc.scalar.activation(sh[:], zrh[:], AF.Sin)
            sh2 = tb(tag="sh2")
            nc.scalar.activation(sh2[:], sh[:], AF.Square)
            t2 = tb(tag="t2")
            nc.scalar.activation(t2[:], t[:], AF.Square)
            u = tb(tag="u")
            nc.gpsimd.tensor_scalar(u[:], t2[:], -1.0, 1.0, ALU.mult, ALU.add)
            ush = tb(tag="ush")
            nc.vector.tensor_mul(ush[:], u[:], sh2[:])
            d = tb(tag="d")
            nc.gpsimd.tensor_scalar(d[:], ush[:], -1.0, 1.0, ALU.mult, ALU.add)
            R = tb(tag="R")
            nc.vector.reciprocal(R[:], d[:])
            su = tb(F16, tag="su")
            nc.gpsimd.tensor_mul(su[:], s2[:], u[:])
            sur = tb(F16, tag="sur")
            nc.vector.tensor_mul(sur[:], su[:], R[:])
            pk_re = pk_pool.tile([128, 256], F16, tag="pk_re",
                                 name=f"pkre{quad}")
            pk_im = pk_pool.tile([128, 256], F16, tag="pk_im",
                                 name=f"pkim{quad}")
            nc.vector.tensor_mul(pk_re[0:64, :], t[:], R[:])
            nc.vector.tensor_copy(pk_im[64:128, :], pk_re[0:64, :])
            nc.gpsimd.tensor_scalar_mul(pk_im[0:64, :], sur[:], 0.5)
            nc.vector.tensor_scalar_mul(pk_re[64:128, :], sur[:], -0.5)
            pk_tiles[quad] = (pk_re, pk_im)

        v_tiles = [None] * 4

        def emit_e2(quad, pool):
            """e2 + v evacuation for one quad (wp-independent)."""
            pk_re, pk_im = pk_tiles[quad]
            ktr4 = ktr4s[quad]
            tagname = "pf" if pool is ps_f else "vx"
            ps_vre = pool.tile([64, 256], F32, tag=tagname,
                               name=f"vre{quad}")
            ps_vim = pool.tile([64, 256], F32, tag=tagname,
                               name=f"vim{quad}")
            for i in range(4):
                col = i * 64
                ks = ktr4[:, i, :]
                nc.tensor.matmul(ps_vre[:, col:col + 64], ks,
                                 pk_re[:, col:col + 64], start=True, stop=True)
                nc.tensor.matmul(ps_vim[:, col:col + 64], ks,
                                 pk_im[:, col:col + 64], start=True, stop=True)
            vre = st_pool.tile([64, 256], F16, tag="vre", name=f"vrs{quad}")
            vim = st_pool.tile([64, 256], F16, tag="vim", name=f"vis{quad}")
            nc.scalar.copy(vre[:], ps_vre[:])
            nc.vector.tensor_copy(vim[:], ps_vim[:])
            v_tiles[quad] = (vre, vim)

        def emit_wmix(quad):
            """wmix + xs for one quad (needs wp)."""
            vre, vim = v_tiles[quad]
            ps_Xre = ps_vx.tile([64, 256], F32, tag="vx", name=f"Xre{quad}")
            ps_Xim = ps_vx.tile([64, 256], F32, tag="vx", name=f"Xim{quad}")
            vre3 = vre[:].rearrange("p (i x) -> p i x", x=64)
            vim3 = vim[:].rearrange("p (i x) -> p i x", x=64)
            Xre3 = ps_Xre[:].rearrange("p (i x) -> p i x", x=64)
            Xim3 = ps_Xim[:].rearrange("p (i x) -> p i x", x=64)
            for x in range(M):
                nc.tensor.matmul(Xre3[:, :, x], wp[:, 0, x, :],
                                 vre3[:, :, x], start=True, stop=False)
                nc.tensor.matmul(Xre3[:, :, x], wp[:, 2, x, :],
                                 vim3[:, :, x], start=False, stop=True)
            for x in range(M):
                nc.tensor.matmul(Xim3[:, :, x], wp[:, 1, x, :],
                                 vre3[:, :, x], start=True, stop=False)
                nc.tensor.matmul(Xim3[:, :, x], wp[:, 0, x, :],
                                 vim3[:, :, x], start=False, stop=True)
            xs = st_pool.tile([128, 256], F16, tag="xs", name=f"xs{quad}")
            nc.vector.tensor_scalar_mul(xs[0:64, :], ps_Xre[:], S_X)
            nc.scalar.activation(xs[64:128, :], ps_Xim[:], AF.Copy,
                                 bias=0.0, scale=S_X)
            xs_tiles[quad] = xs

        def emit_tail_pair(quad, p):
            """X transpose + inverse DFT + output DMA for pair p of quad."""
            xs = xs_tiles[quad]
            pxt = ps_vx.tile([64, 2, 128], F16, tag="vx",
                             name=f"pxt{quad}_{p}")
            for i in (0, 1):
                jq = p * 2 + i
                nc.tensor.transpose(pxt[:, i, :],
                                    xs[:, jq * 64:(jq + 1) * 64], i128[:])
            lhs = st_pool.tile([128, 128], F16, tag="lhs",
                               name=f"lhs{quad}_{p}")
            nc.vector.tensor_copy(lhs[0:64, :], pxt[:, :, 0:64])
            nc.scalar.copy(lhs[64:128, :], pxt[:, :, 64:128])

            ot = ot_pool.tile([128, 2048], F16, tag="ot")
            for c in range(4):
                po = ps_o.tile([128, 512], F32, tag="po",
                               name=f"po{quad}_{p}_{c}")
                nc.tensor.matmul(po[:], lhs[:],
                                 cinv[:, c * 512:(c + 1) * 512],
                                 start=True, stop=True)
                if c % 2 == 0:
                    nc.scalar.copy(ot[:, c * 512:(c + 1) * 512], po[:])
                else:
                    nc.vector.tensor_copy(ot[:, c * 512:(c + 1) * 512], po[:])
            bo = quad * 4 + 2 * p
            nc.scalar.dma_start(OUT[bo:bo + 2, :, :], ot[:])

        # ---------------- program ----------------
        for g in range(2):
            e1_re = ps_e1.tile([64, 512], F32, tag="e1", name=f"e1re{g}")
            e1_im = ps_e1.tile([64, 512], F32, tag="e1", name=f"e1im{g}")
            e1_tiles[g] = (e1_re, e1_im)

        load_ahead(4)

        emit_quad(0)
        load_ahead(8)
        emit_tanh(0)
        emit_quad(1)
        load_ahead(12)
        emit_e2(0, ps_vx)
        emit_wmix(0)
        emit_tanh(1)
        emit_tail_pair(0, 0)
        emit_quad(2)
        load_ahead(16)
        nc.sync.dma_start(wp[:, 0], WP[:, 0])
        nc.sync.dma_start(wp[:, 2], WP[:, 2])
        nc.sync.dma_start(cinv[:], CINV[:])
        nc.sync.dma_start(wp[:, 1], WP[:, 1])
        emit_tail_pair(0, 1)
        emit_e2(1, ps_vx)
        emit_tanh(2)
        emit_wmix(1)
        emit_tail_pair(1, 0)
        emit_tail_pair(1, 1)
        emit_quad(3)
        emit_e2(2, ps_vx)
        emit_tanh(3)
        emit_wmix(2)
        emit_tail_pair(2, 0)
        emit_tail_pair(2, 1)
        emit_e2(3, ps_f)
        emit_wmix(3)
        emit_tail_pair(3, 0)
        emit_tail_pair(3, 1)

    nc.compile()
    return nc


def _host_pack(q, k, w_re, w_im):
    in_maps = []
    for h in range(H):
        qT = np.ascontiguousarray(q[:, :, h, :].transpose(0, 2, 1))
        kT = np.ascontiguousarray(k[:, :, h, :].transpose(0, 2, 1))
        qk = np.concatenate([qT, kT], axis=2)          # [B, L, 128]
        qkh = qk.astype(np.float16)
        qkl = (qk - qkh.astype(np.float32)).astype(np.float16)
        qkhl = np.stack([qkh.reshape(B, 128, NCH, 128),
                         qkl.reshape(B, 128, NCH, 128)], axis=2)

        wre = (w_re[h] * S_W).astype(np.float16)       # [e, o, x]
        wim = (w_im[h] * S_W).astype(np.float16)
        wpk = np.empty((64, 3, M, 64), dtype=np.float16)
        wpk[:, 0] = wre.transpose(0, 2, 1)             # [e, x, o]
        wpk[:, 1] = wim.transpose(0, 2, 1)
        wpk[:, 2] = -wim.transpose(0, 2, 1)
        in_maps.append({
            "qk": np.ascontiguousarray(qkhl),
            "wp": wpk,
        })
    return in_maps


def kernel(q, k, v, w_re, w_im, _trace=False):
    q = np.asarray(q, dtype=np.float32)
    k = np.asarray(k, dtype=np.float32)
    w_re = np.asarray(w_re, dtype=np.float32)
    w_im = np.asarray(w_im, dtype=np.float32)

    if "nc" not in _CACHE:
        _CACHE["nc"] = _build()
    nc = _CACHE["nc"]

    in_maps = _host_pack(q, k, w_re, w_im)
    res = None
    for attempt in range(3):
        try:
            res = run_bass_kernel_spmd(nc, in_maps, list(range(H)),
                                       trace=_trace)
            break
        except Exception:
            if attempt == 2:
                raise
            import time as _time
            _time.sleep(5.0)
    out = np.stack([res.results[h]["out"].astype(np.float32) * S_HOST
                    for h in range(H)], axis=2)
    _CACHE["last_result"] = res
    return np.ascontiguousarray(out)  # [B, E, H, L]
